# revision 17
# baseline (speedup 1.0000x reference)
"""Trainium2 kernel for nn_AdaptivePoolOrGaussian.

Reference computes, per (batch, channel) image X (256x256):
    out = sum_i w_i * (K_i conv X),  w = softmax(alpha)
where the 8 K_i are separable symmetric 11-tap 2D kernels
(5 avg-pools incl. identity + 3 Gaussians), zero-padded "same" convs.

Key identity: all 8 taps vectors are even-symmetric 11-vectors, which
span a 6-dim space, so M = sum_i w_i g_i g_i^T (11x11, PSD) has rank
<= 6. Eigendecompose M = sum_r lam_r q_r q_r^T on the host; then
    out = sum_r lam_r * conv_H(q_r) conv_W(q_r) X     (exact)
Each 1D conv along H/W of a 256-long axis is a banded 256x256 matmul.
On device (per core = one batch element, pure data parallel):
  stage A: Y_r^T = X^T Q_r   via matmul(lhsT=X chunk, rhs=Q_r band cols)
  stage B: out  = sum_r Y_r (lam_r Q_r)  accumulated in PSUM over r.
Band structure lets each k-tile stream only 134 of 256 output columns.
Compute dtype fp16 (PSUM accumulates fp32): rel err ~6e-4.
"""

import numpy as np

import concourse.bass as bass
import concourse.tile as tile
from concourse import mybir
from concourse.bass_utils import run_bass_kernel_spmd

N_CORES = 8
C, H, W = 64, 256, 256
KS, HALF = 11, 5
TRIM = 134              # streamed cols per k-tile (even width, 8B-aligned dst)
TRIM_OFF = (0, 122)     # dst col offset per k-tile; overlap accumulates in PSUM
GC = 8                  # channels per DMA group
EIG_THRESH = 8e-3       # keep eigval if > thresh * max (R=4 for nominal inputs)


def _split_sync_waits(nc: bass.Bass, max_waits: int = 1):
    """walrus in this env encodes at most one sync-wait command per
    instruction; move excess waits onto preceding same-engine NOPs
    (engine queues are in-order, so semantics are preserved)."""
    for f in nc.m.functions:
        for bb in list(f.blocks):
            insts = list(bb.instructions)
            new_insts = []
            changed = False
            for inst in insts:
                si = inst.sync_info
                waits = list(si.on_wait) if si is not None and si.on_wait else []
                if len(waits) > max_waits:
                    extra, keep = waits[:-max_waits], waits[-max_waits:]
                    for w in extra:
                        nop = mybir.InstNoOp(
                            name=nc.get_next_instruction_name(), ins=[], outs=[]
                        )
                        nop.engine = inst.engine
                        nop.sync_info = mybir.SyncInfo(on_wait=[w], on_update=[])
                        nc.register_instruction(nop)
                        new_insts.append(nop)
                    si.on_wait = keep
                    changed = True
                new_insts.append(inst)
            if changed:
                bb.instructions = new_insts


def _host_filters(sigmas: np.ndarray, alpha: np.ndarray):
    """Eigendecompose the combined 2D smoothing operator.

    Returns (qa, qb, R): packed banded filter blocks for stage A / B,
    each (128, 2*R*TRIM) float16.
    """
    al = alpha.astype(np.float64)
    wts = np.exp(al - al.max())
    wts /= wts.sum()

    gs = np.zeros((8, KS))
    gs[0, HALF] = 1.0                                   # identity (k=0)
    for i, k in enumerate((1, 2, 3, 5), start=1):       # avg pools
        gs[i, HALF - k : HALF + k + 1] = 1.0 / (2 * k + 1)
    ax = np.arange(KS, dtype=np.float64) - (KS - 1) / 2.0
    for i in range(3):                                  # gaussians
        s = abs(float(sigmas[i])) + 1e-6
        g = np.exp(-0.5 * (ax / s) ** 2)
        gs[5 + i] = g / g.sum()

    # identity term (i=0) is applied exactly during output evacuation;
    # eigendecompose only the smooth remainder (still rank<=6)
    w0 = float(wts[0])
    M = (gs[1:].T * wts[1:]) @ gs[1:]                   # 11x11 PSD, rank<=6
    lam, V = np.linalg.eigh(M)
    order = np.argsort(lam)[::-1]
    lam, V = lam[order], V[:, order]
    R = max(1, int(np.sum(lam > EIG_THRESH * lam[0])))
    R = min(R, 6)

    def band(q):
        Q = np.zeros((H, H))
        for d in range(-HALF, HALF + 1):
            i = np.arange(max(0, -d), min(H, H - d))
            Q[i, i + d] = q[d + HALF]
        return Q

    def pack(mats):
        out = np.zeros((128, 2 * R * TRIM), np.float16)
        for kt in range(2):
            for r, Q in enumerate(mats):
                blk = Q[kt * 128 : (kt + 1) * 128, TRIM_OFF[kt] : TRIM_OFF[kt] + TRIM]
                out[:, (kt * R + r) * TRIM : (kt * R + r + 1) * TRIM] = blk.astype(
                    np.float16
                )
        return out

    qa = pack([band(V[:, r]) for r in range(R)])
    qb = pack([band(V[:, r] * (lam[r] / w0)) for r in range(R)])
    return qa, qb, R, w0


def _build_nc(R: int) -> bass.Bass:
    nc = bass.Bass()
    x = nc.declare_dram_parameter("x", [C, H, W], mybir.dt.float16, isOutput=False)
    qa = nc.declare_dram_parameter(
        "qa", [128, 2 * R * TRIM], mybir.dt.float16, isOutput=False
    )
    qb = nc.declare_dram_parameter(
        "qb", [128, 2 * R * TRIM], mybir.dt.float16, isOutput=False
    )
    out = nc.declare_dram_parameter("out", [C, H, W], mybir.dt.float16, isOutput=True)

    f16, f32 = mybir.dt.float16, mybir.dt.float32
    n_groups = C // GC

    n_pairs = (R + 1) // 2  # stage-A PSUM tiles hold 2 ranks (2 banks) each

    with tile.TileContext(nc) as tc:
        with (
            tc.tile_pool(name="consts", bufs=1) as consts,
            tc.tile_pool(name="xin", bufs=3) as xin,
            tc.tile_pool(name="ysb", bufs=2 * n_pairs + 1) as ysb,
            tc.tile_pool(name="osb", bufs=2) as osb,
            tc.tile_pool(name="psa", bufs=max(3, n_pairs), space="PSUM") as psa,
            tc.tile_pool(name="pso", bufs=2, space="PSUM") as pso,
        ):
            qa_sb = consts.tile([128, 2 * R * TRIM], f16)
            qb_sb = consts.tile([128, 2 * R * TRIM], f16)
            nc.gpsimd.dma_start(out=qa_sb[:, :], in_=qa[:, :])
            nc.gpsimd.dma_start(out=qb_sb[:, :], in_=qb[:, :])

            # PE clock warm-up: HAM starts the PE at 1.2 GHz and only
            # un-throttles after ~3.4us of sustained activity. Burn dummy
            # matmuls on a zeroed scratch tile while the head DMAs land so
            # the real matmuls start at 2.4 GHz.
            scratch = consts.tile([128, 512], f16, name="scratch")
            nc.gpsimd.memset(scratch[:, :], 0.0)
            warm = psa.tile([128, 1024], f32, name="warm", tag="pa")
            for i in range(20):
                nc.tensor.matmul(
                    warm[:, 0:512],
                    lhsT=scratch[:, 0:128],
                    rhs=scratch[:, 0:512],
                    start=(i == 0),
                    stop=(i == 19),
                )
            eng = [
                nc.scalar.copy,
                lambda out, in_: nc.vector.tensor_copy(out, in_),
            ]
            # input groups: small first so PE starts early; output groups:
            # small last so the final store DMA is short
            import os
            if os.environ.get("UNIFORM_GROUPS"):
                in_sizes = [GC] * (C // GC)
                out_sizes = [GC] * (C // GC)
            else:
                in_sizes = [1, 3, 4] + [GC] * ((C - 8) // GC)
                out_sizes = [GC] * ((C - 8) // GC) + [4, 2, 1, 1]
            def group_map(sizes):
                m, start = {}, 0
                for gi, sz in enumerate(sizes):
                    for off in range(sz):
                        m[start + off] = (gi, off, start, sz)
                    start += sz
                return m
            in_map, out_map = group_map(in_sizes), group_map(out_sizes)

            xgs: dict[int, object] = {}
            ogs: dict[int, object] = {}
            ys_by_c: dict[int, list] = {}

            def stage_a(c):
                g, ci, c0, sz = in_map[c]
                if ci == 0:
                    # x[c, kt*128+p, w] -> xg[p, (dc, kt, w)], cast f32->f16
                    xg = xin.tile([128, sz * 512], f16, name=f"xg{g}", tag="xg")
                    nc.sync.dma_start(
                        out=xg[:, :].rearrange("p (c t w) -> p c t w", c=sz, t=2),
                        in_=x[c0 : c0 + sz].rearrange("c (t p) w -> p c t w", p=128),
                    )
                    xgs[g] = xg
                xg = xgs[g]
                # stage A: Y_r^T = X^T Q_r (contract H on partitions). Rank
                # pair (2j, 2j+1) shares one 2-bank PSUM tile; (kt, mt)
                # outer so consecutive MMs share the stationary X chunk.
                pas = [
                    psa.tile([128, 1024], f32, name=f"pa{j}", tag="pa")
                    for j in range(n_pairs)
                ]
                for kt in range(2):
                    for mt in range(2):
                        base = ci * 512 + kt * 256 + mt * 128
                        lhs = xg[:, base : base + 128]
                        for r in range(R):
                            dst = (r % 2) * 512 + mt * 256 + TRIM_OFF[kt]
                            nc.tensor.matmul(
                                pas[r // 2][:, dst : dst + TRIM],
                                lhsT=lhs,
                                rhs=qa_sb[
                                    :, (kt * R + r) * TRIM : (kt * R + r + 1) * TRIM
                                ],
                                start=(kt == 0 and mt == 0),
                                stop=(kt == 1 and mt == 1),
                            )
                # evacuate PSUM -> SBUF f16, alternating ScalarE/VectorE
                ys = [
                    ysb.tile([128, 1024], f16, name=f"y{j}", tag="y")
                    for j in range(n_pairs)
                ]
                for j in range(n_pairs):
                    width = 1024 if 2 * j + 1 < R else 512
                    e = 0 if j == 0 else (0 if c % 3 == 0 else 1)
                    eng[e](out=ys[j][:, :width], in_=pas[j][:, :width])
                ys_by_c[c] = ys

            def stage_b(c):
                g, ci, c0, sz = out_map[c]
                if ci == 0:
                    ogs[g] = osb.tile([128, sz * 512], f16, name=f"og{g}", tag="og")
                ys = ys_by_c.pop(c)
                # stage B: out = sum_r Y_r (lam_r Q_r)  (contract W)
                po = pso.tile([128, 512], f32)
                for r in range(R):
                    for kt in range(2):
                        for mt in range(2):
                            dst = mt * 256 + TRIM_OFF[kt]
                            src = (r % 2) * 512 + kt * 256 + mt * 128
                            nc.tensor.matmul(
                                po[:, dst : dst + TRIM],
                                lhsT=ys[r // 2][:, src : src + 128],
                                rhs=qb_sb[
                                    :, (kt * R + r) * TRIM : (kt * R + r + 1) * TRIM
                                ],
                                start=(r == 0 and kt == 0 and mt == 0),
                                stop=(r == R - 1 and kt == 1 and mt == 1),
                            )
                gi, cii = in_map[c][0], in_map[c][1]
                nc.vector.tensor_add(
                    ogs[g][:, ci * 512 : (ci + 1) * 512],
                    po[:, :],
                    xgs[gi][:, cii * 512 : (cii + 1) * 512],
                )
                if ci == sz - 1:
                    nc.sync.dma_start(
                        out=out[c0 : c0 + sz].rearrange("c (t p) w -> p c t w", p=128),
                        in_=ogs.pop(g)[:, :].rearrange(
                            "p (c t w) -> p c t w", c=sz, t=2
                        ),
                    )

            # software pipeline: B(c-1) is emitted after A(c), so stage-A
            # evacuations have a full channel of PE work to hide behind
            for c in range(C):
                stage_a(c)
                if c > 0:
                    stage_b(c - 1)
            stage_b(C - 1)
    _split_sync_waits(nc)
    return nc


_NC_CACHE: dict[int, bass.Bass] = {}


def _get_nc(R: int) -> bass.Bass:
    if R not in _NC_CACHE:
        _NC_CACHE[R] = _build_nc(R)
    return _NC_CACHE[R]


def _run(x, sigmas, alpha, trace=False):
    qa, qb, R, w0 = _host_filters(np.asarray(sigmas), np.asarray(alpha))
    # device computes (1/w0) * (sum_r Q_r (w0 X) (lam_r/w0) Q_r + w0 X);
    # scaling X by w0 up front makes the identity term a plain add at evac
    x = (np.asarray(x, dtype=np.float32) * np.float32(w0)).astype(np.float16)
    x = np.ascontiguousarray(x)
    nc = _get_nc(R)
    in_maps = [
        {"x": np.ascontiguousarray(x[i]), "qa": qa, "qb": qb} for i in range(N_CORES)
    ]
    res = run_bass_kernel_spmd(
        nc, in_maps, core_ids=list(range(N_CORES)), trace=trace
    )
    out = np.stack([res.results[i]["out"] for i in range(N_CORES)])
    return out.astype(np.float32), res.exec_time_ns


def kernel(x, sigmas, alpha):
    out, _ = _run(x, sigmas, alpha, trace=False)
    return out


# revision 18
# speedup vs baseline: 1.2276x; 1.2276x over previous
"""Trainium2 kernel for nn_AdaptivePoolOrGaussian.

Reference computes, per (batch, channel) image X (256x256):
    out = sum_i w_i * (K_i conv X),  w = softmax(alpha)
where the 8 K_i are separable symmetric 11-tap 2D kernels
(5 avg-pools incl. identity + 3 Gaussians), zero-padded "same" convs.

Key identity: all 8 taps vectors are even-symmetric 11-vectors, which
span a 6-dim space, so M = sum_i w_i g_i g_i^T (11x11, PSD) has rank
<= 6. Eigendecompose M = sum_r lam_r q_r q_r^T on the host; then
    out = sum_r lam_r * conv_H(q_r) conv_W(q_r) X     (exact)
Each 1D conv along H/W of a 256-long axis is a banded 256x256 matmul.
On device (per core = one batch element, pure data parallel):
  stage A: Y_r^T = X^T Q_r   via matmul(lhsT=X chunk, rhs=Q_r band cols)
  stage B: out  = sum_r Y_r (lam_r Q_r)  accumulated in PSUM over r.
Band structure lets each k-tile stream only 134 of 256 output columns.
Compute dtype fp16 (PSUM accumulates fp32): rel err ~6e-4.
"""

import numpy as np

import concourse.bass as bass
import concourse.tile as tile
from concourse import mybir
from concourse.bass_utils import run_bass_kernel_spmd

N_CORES = 8
C, H, W = 64, 256, 256
KS, HALF = 11, 5
TRIM = 134              # streamed cols per k-tile (even width, 8B-aligned dst)
TRIM_OFF = (0, 122)     # dst col offset per k-tile; overlap accumulates in PSUM
GC = 8                  # channels per DMA group
EIG_THRESH = 8e-3       # keep eigval if > thresh * max (R=4 for nominal inputs)


def _split_sync_waits(nc: bass.Bass, max_waits: int = 1):
    """walrus in this env encodes at most one sync-wait command per
    instruction; move excess waits onto preceding same-engine NOPs
    (engine queues are in-order, so semantics are preserved)."""
    for f in nc.m.functions:
        for bb in list(f.blocks):
            insts = list(bb.instructions)
            new_insts = []
            changed = False
            for inst in insts:
                si = inst.sync_info
                waits = list(si.on_wait) if si is not None and si.on_wait else []
                if len(waits) > max_waits:
                    extra, keep = waits[:-max_waits], waits[-max_waits:]
                    for w in extra:
                        nop = mybir.InstNoOp(
                            name=nc.get_next_instruction_name(), ins=[], outs=[]
                        )
                        nop.engine = inst.engine
                        nop.sync_info = mybir.SyncInfo(on_wait=[w], on_update=[])
                        nc.register_instruction(nop)
                        new_insts.append(nop)
                    si.on_wait = keep
                    changed = True
                new_insts.append(inst)
            if changed:
                bb.instructions = new_insts


def _host_filters(sigmas: np.ndarray, alpha: np.ndarray):
    """Eigendecompose the combined 2D smoothing operator.

    Returns (qa, qb, R): packed banded filter blocks for stage A / B,
    each (128, 2*R*TRIM) float16.
    """
    al = alpha.astype(np.float64)
    wts = np.exp(al - al.max())
    wts /= wts.sum()

    gs = np.zeros((8, KS))
    gs[0, HALF] = 1.0                                   # identity (k=0)
    for i, k in enumerate((1, 2, 3, 5), start=1):       # avg pools
        gs[i, HALF - k : HALF + k + 1] = 1.0 / (2 * k + 1)
    ax = np.arange(KS, dtype=np.float64) - (KS - 1) / 2.0
    for i in range(3):                                  # gaussians
        s = abs(float(sigmas[i])) + 1e-6
        g = np.exp(-0.5 * (ax / s) ** 2)
        gs[5 + i] = g / g.sum()

    # identity term (i=0) is applied exactly during output evacuation;
    # eigendecompose only the smooth remainder (still rank<=6)
    w0 = float(wts[0])
    M = (gs[1:].T * wts[1:]) @ gs[1:]                   # 11x11 PSD, rank<=6
    lam, V = np.linalg.eigh(M)
    order = np.argsort(lam)[::-1]
    lam, V = lam[order], V[:, order]
    R = max(1, int(np.sum(lam > EIG_THRESH * lam[0])))
    R = min(R, 6)

    def band(q):
        Q = np.zeros((H, H))
        for d in range(-HALF, HALF + 1):
            i = np.arange(max(0, -d), min(H, H - d))
            Q[i, i + d] = q[d + HALF]
        return Q

    def pack(mats):
        out = np.zeros((128, 2 * R * TRIM), np.float16)
        for kt in range(2):
            for r, Q in enumerate(mats):
                blk = Q[kt * 128 : (kt + 1) * 128, TRIM_OFF[kt] : TRIM_OFF[kt] + TRIM]
                out[:, (kt * R + r) * TRIM : (kt * R + r + 1) * TRIM] = blk.astype(
                    np.float16
                )
        return out

    qa = pack([band(V[:, r]) for r in range(R)])
    qb = pack([band(V[:, r] * (lam[r] / w0)) for r in range(R)])
    return qa, qb, R, w0


def _build_nc(R: int) -> bass.Bass:
    nc = bass.Bass()
    x = nc.declare_dram_parameter("x", [C, H, W], mybir.dt.float16, isOutput=False)
    qa = nc.declare_dram_parameter(
        "qa", [128, 2 * R * TRIM], mybir.dt.float16, isOutput=False
    )
    qb = nc.declare_dram_parameter(
        "qb", [128, 2 * R * TRIM], mybir.dt.float16, isOutput=False
    )
    out = nc.declare_dram_parameter("out", [C, H, W], mybir.dt.float16, isOutput=True)

    f16, f32 = mybir.dt.float16, mybir.dt.float32
    n_groups = C // GC

    n_pairs = (R + 1) // 2  # stage-A PSUM tiles hold 2 ranks (2 banks) each

    with tile.TileContext(nc) as tc:
        with (
            tc.tile_pool(name="consts", bufs=1) as consts,
            tc.tile_pool(name="xin", bufs=3) as xin,
            tc.tile_pool(name="ysb", bufs=2 * n_pairs + 1) as ysb,
            tc.tile_pool(name="osb", bufs=2) as osb,
            tc.tile_pool(name="psa", bufs=max(3, n_pairs), space="PSUM") as psa,
            tc.tile_pool(name="pso", bufs=2, space="PSUM") as pso,
        ):
            qa_sb = consts.tile([128, 2 * R * TRIM], f16)
            qb_sb = consts.tile([128, 2 * R * TRIM], f16)
            nc.gpsimd.dma_start(out=qa_sb[:, :], in_=qa[:, :])
            nc.gpsimd.dma_start(out=qb_sb[:, :], in_=qb[:, :])

            # PE clock warm-up: HAM starts the PE at 1.2 GHz and only
            # un-throttles after ~3.4us of sustained activity. Burn dummy
            # matmuls on a zeroed scratch tile while the head DMAs land so
            # the real matmuls start at 2.4 GHz.
            scratch = consts.tile([128, 512], f16, name="scratch")
            nc.gpsimd.memset(scratch[:, :], 0.0)
            warm = psa.tile([128, 1024], f32, name="warm", tag="pa")
            for i in range(26):
                nc.tensor.matmul(
                    warm[:, 0:128],
                    lhsT=scratch[:, 0:128],
                    rhs=scratch[:, 0:128],
                    start=(i == 0),
                    stop=(i == 25),
                )
            eng = [
                nc.scalar.copy,
                lambda out, in_: nc.vector.tensor_copy(out, in_),
            ]
            # input groups: small first so PE starts early; output groups:
            # small last so the final store DMA is short
            import os
            if os.environ.get("UNIFORM_GROUPS"):
                in_sizes = [GC] * (C // GC)
                out_sizes = [GC] * (C // GC)
            else:
                in_sizes = [1, 3, 4] + [GC] * ((C - 8) // GC)
                out_sizes = [GC] * ((C - 8) // GC) + [4, 2, 1, 1]
            def group_map(sizes):
                m, start = {}, 0
                for gi, sz in enumerate(sizes):
                    for off in range(sz):
                        m[start + off] = (gi, off, start, sz)
                    start += sz
                return m
            in_map, out_map = group_map(in_sizes), group_map(out_sizes)

            xgs: dict[int, object] = {}
            ogs: dict[int, object] = {}
            ys_by_c: dict[int, list] = {}

            def stage_a(c):
                g, ci, c0, sz = in_map[c]
                if ci == 0:
                    # x[c, kt*128+p, w] -> xg[p, (dc, kt, w)], cast f32->f16
                    xg = xin.tile([128, sz * 512], f16, name=f"xg{g}", tag="xg")
                    nc.sync.dma_start(
                        out=xg[:, :].rearrange("p (c t w) -> p c t w", c=sz, t=2),
                        in_=x[c0 : c0 + sz].rearrange("c (t p) w -> p c t w", p=128),
                    )
                    xgs[g] = xg
                xg = xgs[g]
                # stage A: Y_r^T = X^T Q_r (contract H on partitions). Rank
                # pair (2j, 2j+1) shares one 2-bank PSUM tile; (kt, mt)
                # outer so consecutive MMs share the stationary X chunk.
                pas = [
                    psa.tile([128, 1024], f32, name=f"pa{j}", tag="pa")
                    for j in range(n_pairs)
                ]
                for kt in range(2):
                    for mt in range(2):
                        base = ci * 512 + kt * 256 + mt * 128
                        lhs = xg[:, base : base + 128]
                        for r in range(R):
                            dst = (r % 2) * 512 + mt * 256 + TRIM_OFF[kt]
                            nc.tensor.matmul(
                                pas[r // 2][:, dst : dst + TRIM],
                                lhsT=lhs,
                                rhs=qa_sb[
                                    :, (kt * R + r) * TRIM : (kt * R + r + 1) * TRIM
                                ],
                                start=(kt == 0 and mt == 0),
                                stop=(kt == 1 and mt == 1),
                            )
                # evacuate PSUM -> SBUF f16, alternating ScalarE/VectorE
                ys = [
                    ysb.tile([128, 1024], f16, name=f"y{j}", tag="y")
                    for j in range(n_pairs)
                ]
                for j in range(n_pairs):
                    width = 1024 if 2 * j + 1 < R else 512
                    e = 0 if j == 0 else (0 if c % 3 == 0 else 1)
                    eng[e](out=ys[j][:, :width], in_=pas[j][:, :width])
                ys_by_c[c] = ys

            def stage_b(c):
                g, ci, c0, sz = out_map[c]
                if ci == 0:
                    ogs[g] = osb.tile([128, sz * 512], f16, name=f"og{g}", tag="og")
                ys = ys_by_c.pop(c)
                # stage B: out = sum_r Y_r (lam_r Q_r)  (contract W)
                po = pso.tile([128, 512], f32)
                for r in range(R):
                    for kt in range(2):
                        for mt in range(2):
                            dst = mt * 256 + TRIM_OFF[kt]
                            src = (r % 2) * 512 + kt * 256 + mt * 128
                            nc.tensor.matmul(
                                po[:, dst : dst + TRIM],
                                lhsT=ys[r // 2][:, src : src + 128],
                                rhs=qb_sb[
                                    :, (kt * R + r) * TRIM : (kt * R + r + 1) * TRIM
                                ],
                                start=(r == 0 and kt == 0 and mt == 0),
                                stop=(r == R - 1 and kt == 1 and mt == 1),
                            )
                gi, cii = in_map[c][0], in_map[c][1]
                nc.vector.tensor_add(
                    ogs[g][:, ci * 512 : (ci + 1) * 512],
                    po[:, :],
                    xgs[gi][:, cii * 512 : (cii + 1) * 512],
                )
                if ci == sz - 1:
                    nc.sync.dma_start(
                        out=out[c0 : c0 + sz].rearrange("c (t p) w -> p c t w", p=128),
                        in_=ogs.pop(g)[:, :].rearrange(
                            "p (c t w) -> p c t w", c=sz, t=2
                        ),
                    )

            # software pipeline: B(c-1) is emitted after A(c), so stage-A
            # evacuations have a full channel of PE work to hide behind
            for c in range(C):
                stage_a(c)
                if c > 0:
                    stage_b(c - 1)
            stage_b(C - 1)
    _split_sync_waits(nc)
    return nc


_NC_CACHE: dict[int, bass.Bass] = {}


def _get_nc(R: int) -> bass.Bass:
    if R not in _NC_CACHE:
        _NC_CACHE[R] = _build_nc(R)
    return _NC_CACHE[R]


def _run(x, sigmas, alpha, trace=False):
    qa, qb, R, w0 = _host_filters(np.asarray(sigmas), np.asarray(alpha))
    # device computes (1/w0) * (sum_r Q_r (w0 X) (lam_r/w0) Q_r + w0 X);
    # scaling X by w0 up front makes the identity term a plain add at evac
    x = (np.asarray(x, dtype=np.float32) * np.float32(w0)).astype(np.float16)
    x = np.ascontiguousarray(x)
    nc = _get_nc(R)
    in_maps = [
        {"x": np.ascontiguousarray(x[i]), "qa": qa, "qb": qb} for i in range(N_CORES)
    ]
    res = run_bass_kernel_spmd(
        nc, in_maps, core_ids=list(range(N_CORES)), trace=trace
    )
    out = np.stack([res.results[i]["out"] for i in range(N_CORES)])
    return out.astype(np.float32), res.exec_time_ns


def kernel(x, sigmas, alpha):
    out, _ = _run(x, sigmas, alpha, trace=False)
    return out
